# revision 63
# baseline (speedup 1.0000x reference)
"""KeypointFlowLoss Trainium2 kernel.

The loss only reads each flow at the K keypoint pixels that the reference
scatters into the ground-truth flow image (every other pixel has gt == 0 and
mask == 0), so instead of streaming 5 x [16,2,512,512] f32 from HBM we gather
exactly the needed pixels with indirect DMA and reduce on-chip.

Sharding: data-parallel over the batch dim -- core c owns batches
[2c, 2c+2). Each core ships its [34, 6] per-keypoint columns ([5 masked
EPE values, mask]); the host folds the keypoint sum into the cross-core
all-reduce and applies the weighted division.

The critical path is a 3-deep serial DMA chain (kps in -> gather -> result
out) whose fixed latencies dominate, so the kernel is structured to keep
that chain minimal:
  * kps is passed host-transposed as [NP, 4] rows so ONE simple DMA loads it
  * the 5 flows are passed host-interleaved as [B,H,W,2,5] so ONE indirect
    DMA with one offset per keypoint (coefficient 10, 40 B contiguous runs)
    gathers every needed value at once
  * the kt -> gather-offset chain is 2 fused DVE ops in int32; everything
    else (batch offset column, displacement, mask) is computed off the
    critical path while DMAs are in flight
  * sqrt and masking fuse into one ACT op: sqrt(s * mask) == mask * sqrt(s)
  * the output store is a PREPARED dma_scatter_add: descriptors are
    generated on Pool during the gather flight (the source read defers to
    the trigger), the destination rows are pre-zeroed by an overlapped
    plain store, and after the sqrt only a trigger + the 8.5 KB transfer
    remain -- the store's full DMA pipeline is off the critical path
  * the TileContext epilogue replaces drain + barrier + sem clears + barrier
    with: each engine signals completion at sequencer level, and Pool waits
    every DMA-completion and engine-tick semaphore on single-wait
    EventSemaphores (the semaphore set covers all outstanding work, making
    the engine drains redundant; clears are unnecessary because each NEFF
    execution starts from freshly initialized semaphore state)
"""

import numpy as np

import concourse.bacc as bacc
import concourse.bass as bass
import concourse.mybir as mybir
import concourse.tile as tile
from concourse.bass import IndirectOffsetOnAxis
from concourse.bass_utils import run_bass_kernel_spmd

B, CH, H, W = 16, 2, 512, 512
K = 17
NF = 5
NCORES = 8
BL = B // NCORES          # batches per core
NP = BL * K               # keypoints per core
NPIX = BL * CH * H * W    # pixels per core (per channel-plane view)
GAMMA = 0.8
LOSS_WEIGHT = 1.0

F32 = mybir.dt.float32
I32 = mybir.dt.int32
I16 = mybir.dt.int16
OUTW = 64   # output row padded to 64 f32 = 256 B (scatter-add stride contract)

_PROGRAM = None
_RUN_KWARGS = {}      # test harness can set {"trace": True} to profile
_LAST_RESULTS = None


class _TrimmedTileContext(tile.TileContext):
    """TileContext with a minimal single-shot epilogue.

    The standard epilogue is drain -> barrier -> sem clears -> barrier,
    which serializes on the full DMA pipeline twice. Here every engine just
    signals at sequencer level and Pool waits for the complete wait-clock
    (every DMA-completion and engine-tick semaphore, one per
    EventSemaphore), which subsumes what the drains guaranteed. The sem
    clears and release round only matter when more program follows or the
    loaded NEFF re-executes with retained semaphore state; this kernel ends
    right after, and each execution starts from fresh semaphore state
    (verified by repeated in-process calls).
    """

    def _drain_and_barrier(self, tick_clock, wait_clock):
        # gather-only half barrier: every engine signals completion; Pool
        # waits for all of them AND for every outstanding DMA-completion /
        # engine-tick semaphore (the wait-clock), one per single-wait
        # EventSemaphore. The release round is dropped -- the other engines
        # halt right after signalling, so they have nothing to wait for.
        scratch = mybir.InstEventSemaphore(
            name="epilogue_waits", engine=mybir.EngineType.Pool, ins=[], outs=[],
            sync_info=mybir.SyncInfo(on_wait=[], on_update=[]))
        wait_clock.add_sem_waits(
            scratch, tile.ScopedClock({None: tick_clock.global_clock})
        )
        final_waits = list(scratch.sync_info.on_wait) if scratch.sync_info else []
        for inst in self.nc._multi_engine_barrier_insts(list(self.nc.engines)):
            si = inst.sync_info
            if si is not None and any(
                u.ant_name.endswith("_release") for u in si.on_update
            ):
                continue
            if si is not None and any(
                w.ant_name.endswith("_gather") for w in si.on_wait
            ):
                for i, w in enumerate(final_waits):
                    self.nc.engines[inst.engine].add_instruction(
                        mybir.InstEventSemaphore(
                            name=f"epilogue_wait_{i}",
                            engine=mybir.EngineType.Pool, ins=[], outs=[],
                            sync_info=mybir.SyncInfo(on_wait=[w], on_update=[])))
            if type(inst).__name__ == "InstDrain":
                # the wait-clock covers every DMA-completion and engine-tick
                # semaphore, so a sequencer-level signal is enough here; a
                # full engine drain would serialize on the DMA pipeline.
                if si is None or not si.on_update:
                    continue
                inst = mybir.InstEventSemaphore(
                    name=f"epilogue_sig_{inst.engine}", engine=inst.engine,
                    ins=[], outs=[],
                    sync_info=mybir.SyncInfo(on_wait=[], on_update=list(si.on_update)))
            self.nc.engines[inst.engine].add_instruction(inst)
        popped = self.nc._tile_sem_poison_stack.pop()
        assert popped is self._sem_poison


def _view3(ap, inner):
    """View a [P, c*inner] contiguous AP as [P, c, inner]."""
    total = ap.ap[1][1]
    return bass.AP(
        ap.tensor, ap.offset, [list(ap.ap[0]), [inner, total // inner], [1, inner]]
    )


def _build_program():
    nc = bacc.Bacc(None, target_bir_lowering=False)

    # host-interleaved flows [BL,H,W,CH,NF]: all 2*NF values of a pixel are
    # one contiguous run, so a single indirect gather with one offset per
    # keypoint (coefficient CH*NF) reads everything.
    flows = nc.dram_tensor("flows", [BL * H * W, CH * NF], F32, kind="ExternalInput")
    # host-transposed keypoints: row (b*K + k) = [x0, y0, x1, y1]
    kps = nc.dram_tensor("kps", [NP, 4], I32, kind="ExternalInput")
    out = nc.dram_tensor("out", [NP, OUTW], F32, kind="ExternalOutput")

    with _TrimmedTileContext(nc) as tc:
        with tc.tile_pool(name="sbuf", bufs=1) as sb:
            kt = sb.tile([NP, 4], I32)
            nc.gpsimd.dma_start(out=kt[:], in_=kps[:])

            # ---- no-dependency work, runs while the kps DMA is in flight ----
            # bc[p] = (b >= 1) * H*W  (batch pixel base, b = p // K for BL=2)
            pidx = sb.tile([NP, 1], I32)
            nc.gpsimd.iota(pidx[:], pattern=[[0, 1]], base=0, channel_multiplier=1)
            bc = sb.tile([NP, 1], I32)
            nc.vector.tensor_scalar(out=bc[:], in0=pidx[:], scalar1=K - 1,
                                    scalar2=H * W, op0=mybir.AluOpType.is_gt,
                                    op1=mybir.AluOpType.mult)
            # ---- critical chain: gather offsets from keypoint coords ----
            # xb = x0 + bc ; offs = y0 * W + xb  (pixel index within the core)
            with tc.high_priority():
                xb = sb.tile([NP, 1], I32)
                nc.vector.tensor_tensor(out=xb[:], in0=kt[:, 0:1], in1=bc[:],
                                        op=mybir.AluOpType.add)
                offs = sb.tile([NP, 1], I32)
                nc.vector.scalar_tensor_tensor(out=offs[:], in0=kt[:, 1:2],
                                               scalar=W, in1=xb[:],
                                               op0=mybir.AluOpType.mult,
                                               op1=mybir.AluOpType.add)

                # one indirect gather: row p = [f0..f4 @ ch0 | f0..f4 @ ch1]
                g = sb.tile([NP, 2 * NF], F32)
                nc.gpsimd.indirect_dma_start(
                    out=g[:],
                    out_offset=None,
                    in_=bass.AP(flows, 0, [[CH * NF, BL * H * W], [1, CH * NF]]),
                    in_offset=IndirectOffsetOnAxis(ap=offs[:], axis=0),
                )

            # ---- in-flight work: displacement + mask (not on critical path) ----
            kf = sb.tile([NP, 4], F32)
            nc.vector.tensor_copy(out=kf[:], in_=kt[:])  # int -> float, exact
            disp = sb.tile([NP, 2], F32)
            nc.vector.tensor_tensor(out=disp[:], in0=kf[:, 2:4], in1=kf[:, 0:2],
                                    op=mybir.AluOpType.subtract)
            dsq = sb.tile([NP, 2], F32)
            nc.vector.tensor_tensor(out=dsq[:], in0=disp[:], in1=disp[:],
                                    op=mybir.AluOpType.mult)
            r2 = sb.tile([NP, 1], F32)
            nc.vector.tensor_tensor(out=r2[:], in0=dsq[:, 0:1], in1=dsq[:, 1:2],
                                    op=mybir.AluOpType.add)
            # all kps are in [0, W) by construction, so gt != 0 is the only
            # mask condition (matches reference: norm(gt) > 0)
            mask = sb.tile([NP, 1], F32)
            nc.vector.tensor_scalar(out=mask[:], in0=r2[:], scalar1=0.0,
                                    scalar2=None, op0=mybir.AluOpType.is_gt)
            # scatter-add source: row p = [EPE f0..f4, mask, 0...]; 128
            # partitions and 256 B rows per the dma_scatter_add contract
            vp = sb.tile([128, OUTW], F32)
            nc.vector.memset(vp[:], 0.0)
            nc.vector.tensor_copy(out=vp[0:NP, NF:NF + 1], in_=mask[:])

            # identity indices, int16, wrapped [16 partitions x 3 cols]:
            # idx[i] = i for i < NP, -1 beyond (ignored); unused partitions 0.
            # The clamp runs on the otherwise-idle Pool engine: keep the iota
            # value where 33 - p - 16*s >= 0, else fill -1.
            idxs = sb.tile([128, 3], I16)
            nc.gpsimd.memset(idxs[:], 0)
            nc.gpsimd.iota(idxs[0:16, :], pattern=[[16, 3]], base=0,
                           channel_multiplier=1)
            nc.gpsimd.affine_select(out=idxs[0:16, :], in_=idxs[0:16, :],
                                    pattern=[[-16, 3]], base=NP - 1,
                                    channel_multiplier=-1,
                                    compare_op=mybir.AluOpType.is_ge,
                                    fill=-1.0)

            # pre-zero the output rows (scatter ADDS); fully overlapped
            zt = sb.tile([NP, OUTW], F32)
            nc.vector.memset(zt[:], 0.0)
            nc.sync.dma_start(out=out[:], in_=zt[:])

            # ---- post-gather: EPE columns ----
            u = sb.tile([NP, 2 * NF], F32)   # u = g - disp (disp bcast over f)
            dispB = bass.AP(disp[:].tensor, disp[:].offset,
                            [list(disp[:].ap[0]), [1, 2], [0, NF]])
            nc.vector.tensor_tensor(out=_view3(u[:], NF), in0=_view3(g[:], NF),
                                    in1=dispB, op=mybir.AluOpType.subtract)
            d2 = sb.tile([NP, 2 * NF], F32)
            nc.vector.tensor_tensor(out=d2[:], in0=u[:], in1=u[:],
                                    op=mybir.AluOpType.mult)
            s5 = sb.tile([NP, NF], F32)
            nc.vector.tensor_tensor(out=s5[:], in0=d2[:, 0:NF], in1=d2[:, NF:2 * NF],
                                    op=mybir.AluOpType.add)
            # ACT Sqrt is table-approximated (~1e-5 rel) -- well within the
            # 2e-2 gate. mask in {0,1} so sqrt(s*mask) == mask*sqrt(s).
            nc.scalar.activation(out=vp[0:NP, 0:NF], in_=s5[:],
                                 func=mybir.ActivationFunctionType.Sqrt,
                                 scale=mask[:])

            # ship the per-keypoint rows via a PREPARED scatter-add: the
            # descriptors are generated during the gather flight (the source
            # read defers to the trigger), so after the sqrt only a cheap
            # trigger + the actual 8.5 KB transfer remain. The host folds the
            # keypoint sum into the cross-core reduction it already does.
            dma_sem = nc.alloc_semaphore("out_scatter_dma")
            nc.gpsimd.dma_scatter_add(
                out[:],
                bass.AP(vp[:].tensor, vp[:].offset,
                        [[OUTW, 128], [OUTW, 1], [1, OUTW]]),
                idxs[:],
                NP, NP, OUTW,
                prepare_only=True,
                sem=dma_sem,
            )
            trig = nc.gpsimd.trigger_dma(count=None)
            wait = nc.gpsimd.wait_ge(dma_sem, 16)
            tile.add_dep_helper(wait.ins, trig.ins,
                                reason="dma sem wait follows trigger")

    nc.finalize()
    return nc


def _get_program():
    global _PROGRAM
    if _PROGRAM is None:
        _PROGRAM = _build_program()
    return _PROGRAM


def kernel(**inputs):
    flows = [np.asarray(inputs[f"flow{i}"], dtype=np.float32) for i in range(NF)]
    kps = np.asarray(inputs["kps"], dtype=np.int32)

    nc = _get_program()

    # [B,H,W,CH,NF]: pixel (b,y,x) holds [c0f0..c0f4, c1f0..c1f4] contiguous.
    # One strided pass per flow into a preallocated buffer.
    fl_all = np.empty((B, H, W, CH, NF), dtype=np.float32)
    for f in range(NF):
        fl_all[..., f] = flows[f].transpose(0, 2, 3, 1)
    fl_all = fl_all.reshape(B, H * W, CH * NF)

    in_maps = []
    for c in range(NCORES):
        sl = slice(c * BL, (c + 1) * BL)
        m = {
            "flows": fl_all[sl].reshape(BL * H * W, CH * NF),
            "kps": np.ascontiguousarray(
                kps[sl].transpose(0, 2, 1, 3).reshape(NP, 4)),
        }
        in_maps.append(m)

    results = run_bass_kernel_spmd(nc, in_maps, core_ids=list(range(NCORES)),
                                   **_RUN_KWARGS)
    globals()["_LAST_RESULTS"] = results

    total = np.zeros(NF + 1, dtype=np.float64)
    for r in results.results:
        total += r["out"].reshape(NP, OUTW)[:, :NF + 1].astype(np.float64).sum(axis=0)

    sums, cnt = total[:NF], total[NF]
    weights = GAMMA ** np.arange(NF - 1, -1, -1, dtype=np.float64)
    means = sums / cnt
    loss = np.float32(np.sum(weights * means) * LOSS_WEIGHT)
    return np.asarray(loss, dtype=np.float32)
